# revision 3
# baseline (speedup 1.0000x reference)
"""ASTGCN block kernel for 8 Trainium2 NeuronCores.

Pure data parallel: batch dim B=4096 sharded 512-per-core across the 8
cores; all params replicated. The per-core computation is expressed in
JAX and compiled for the NeuronCores through the PJRT backend (shard_map
over an 8-device mesh), so all compute runs on the trn2 devices.
"""

import numpy as np

B, N, F_IN, T = 4096, 38, 64, 5
K, C_CHEB, C_TIME = 3, 64, 64
EPS = 1e-5
NCORES = 8

_cache = {}


def _get_compiled():
    if "fn" in _cache:
        return _cache["fn"]
    import jax
    import jax.numpy as jnp
    from jax.sharding import Mesh, PartitionSpec as P
    from jax.experimental.shard_map import shard_map

    devs = jax.devices()
    nd = NCORES
    while nd > 1 and (len(devs) < nd or B % nd != 0):
        nd //= 2
    devs = devs[:nd]
    mesh = Mesh(np.array(devs), ("x",))

    BF = jnp.bfloat16
    F32 = jnp.float32

    def block(x, cheb, U1, U2, U3, b_e, V_e, W1, W2, W3, b_s, V_s,
              Theta, W_time, b_time, W_res, b_res, gamma, beta):
        x16 = x.astype(BF)
        # temporal attention (projections of full x in bf16, f32 accumulate)
        lhs = jnp.einsum('bnft,n->btf', x16, U1.astype(BF),
                         preferred_element_type=F32)
        lhs = jnp.einsum('btf,fn->btn', lhs, U2)
        rhs = jnp.einsum('f,bnft->bnt', U3.astype(BF), x16,
                         preferred_element_type=F32)
        prod = jnp.einsum('btn,bns->bts', lhs, rhs)
        E = jnp.einsum('btj,ij->bti', jax.nn.sigmoid(prod + b_e), V_e)
        t_at = jax.nn.softmax(E, axis=1)
        x_tat = jnp.einsum('bnft,bts->bnfs', x16, t_at.astype(BF),
                           preferred_element_type=F32)

        # spatial attention
        sl = jnp.einsum('bnft,t->bnf', x_tat, W1)
        sl = jnp.einsum('bnf,ft->bnt', sl, W2)
        sr = jnp.einsum('f,bmft->bmt', W3, x_tat)
        sp = jnp.einsum('bnt,bmt->bnm', sl, sr)
        S = jnp.einsum('nk,bkm->bnm', V_s, jax.nn.sigmoid(sp + b_s))
        s_at = jax.nn.softmax(S, axis=1)

        # K-order chebyshev conv with spatial attention
        tk_at = (cheb[None] * s_at[:, None]).astype(BF)
        rhs_g = jnp.einsum('bkmn,bmft->bknft', tk_at, x16,
                           preferred_element_type=F32).astype(BF)
        gcn = jax.nn.relu(jnp.einsum('bknft,kfo->bnot', rhs_g,
                                     Theta.astype(BF),
                                     preferred_element_type=F32))

        # temporal conv (1,3) pad (0,1) on (B, C, N, T)
        g = jnp.transpose(gcn, (0, 2, 1, 3)).astype(BF)
        tco = jax.lax.conv_general_dilated(
            g, W_time.astype(BF), window_strides=(1, 1),
            padding=((0, 0), (1, 1)),
            dimension_numbers=('NCHW', 'OIHW', 'NCHW'),
            preferred_element_type=F32) \
            + b_time[None, :, None, None]

        # 1x1 residual conv
        xr = jnp.transpose(x16, (0, 2, 1, 3))
        res = jax.lax.conv_general_dilated(
            xr, W_res.astype(BF), window_strides=(1, 1),
            padding=((0, 0), (0, 0)),
            dimension_numbers=('NCHW', 'OIHW', 'NCHW'),
            preferred_element_type=F32) \
            + b_res[None, :, None, None]

        h = jax.nn.relu(res + tco)
        hp = jnp.transpose(h, (0, 3, 2, 1))
        mu = jnp.mean(hp, axis=-1, keepdims=True)
        var = jnp.mean(jnp.square(hp - mu), axis=-1, keepdims=True)
        ln = gamma * (hp - mu) * jax.lax.rsqrt(var + EPS) + beta
        return jnp.transpose(ln, (0, 2, 3, 1))

    pspec_x = P("x")          # shard batch dim
    pspec_rep = P()           # replicated params
    in_specs = (pspec_x,) + (pspec_rep,) * 18
    fn = jax.jit(
        shard_map(block, mesh=mesh, in_specs=in_specs, out_specs=pspec_x)
    )
    _cache["fn"] = fn
    return fn


def kernel(x, cheb, U1, U2, U3, b_e, V_e, W1, W2, W3, b_s, V_s,
           Theta, W_time, b_time, W_res, b_res, gamma, beta):
    import jax.numpy as jnp

    fn = _get_compiled()
    args = [x, cheb, U1, U2, U3, b_e, V_e, W1, W2, W3, b_s, V_s,
            Theta, W_time, b_time, W_res, b_res, gamma, beta]
    args = [jnp.asarray(np.asarray(a), jnp.float32) for a in args]
    out = fn(*args)
    return np.asarray(out, dtype=np.float32)


# revision 4
# speedup vs baseline: 2.3546x; 2.3546x over previous
"""ASTGCN block kernel for 8 Trainium2 NeuronCores.

Pure data parallel: batch dim B=4096 sharded 512-per-core across the 8
cores; all params replicated. The per-core computation is expressed in
JAX and compiled for the NeuronCores through the PJRT backend (shard_map
over an 8-device mesh), so all compute runs on the trn2 devices.
"""

import numpy as np

B, N, F_IN, T = 4096, 38, 64, 5
K, C_CHEB, C_TIME = 3, 64, 64
EPS = 1e-5
NCORES = 8

_cache = {}


def _get_compiled():
    if "fn" in _cache:
        return _cache["fn"]
    import jax
    import jax.numpy as jnp
    from jax.sharding import Mesh, PartitionSpec as P
    from jax.experimental.shard_map import shard_map

    devs = jax.devices()
    nd = NCORES
    while nd > 1 and (len(devs) < nd or B % nd != 0):
        nd //= 2
    devs = devs[:nd]
    mesh = Mesh(np.array(devs), ("x",))

    def block(x, cheb, U1, U2, U3, b_e, V_e, W1, W2, W3, b_s, V_s,
              Theta, W_time, b_time, W_res, b_res, gamma, beta):
        # temporal attention
        lhs = jnp.einsum('bnft,n->btf', x, U1)
        lhs = jnp.einsum('btf,fn->btn', lhs, U2)
        rhs = jnp.einsum('f,bnft->bnt', U3, x)
        prod = jnp.einsum('btn,bns->bts', lhs, rhs)
        E = jnp.einsum('btj,ij->bti', jax.nn.sigmoid(prod + b_e), V_e)
        t_at = jax.nn.softmax(E, axis=1)
        x_tat = jnp.einsum('bnft,bts->bnfs', x, t_at)

        # spatial attention
        sl = jnp.einsum('bnft,t->bnf', x_tat, W1)
        sl = jnp.einsum('bnf,ft->bnt', sl, W2)
        sr = jnp.einsum('f,bmft->bmt', W3, x_tat)
        sp = jnp.einsum('bnt,bmt->bnm', sl, sr)
        S = jnp.einsum('nk,bkm->bnm', V_s, jax.nn.sigmoid(sp + b_s))
        s_at = jax.nn.softmax(S, axis=1)

        # K-order chebyshev conv with spatial attention
        tk_at = cheb[None] * s_at[:, None]
        rhs_g = jnp.einsum('bkmn,bmft->bknft', tk_at, x)
        gcn = jax.nn.relu(jnp.einsum('bknft,kfo->bnot', rhs_g, Theta))

        # temporal conv (1,3) pad (0,1) on (B, C, N, T)
        g = jnp.transpose(gcn, (0, 2, 1, 3))
        tco = jax.lax.conv_general_dilated(
            g, W_time, window_strides=(1, 1), padding=((0, 0), (1, 1)),
            dimension_numbers=('NCHW', 'OIHW', 'NCHW')) \
            + b_time[None, :, None, None]

        # 1x1 residual conv
        xr = jnp.transpose(x, (0, 2, 1, 3))
        res = jax.lax.conv_general_dilated(
            xr, W_res, window_strides=(1, 1), padding=((0, 0), (0, 0)),
            dimension_numbers=('NCHW', 'OIHW', 'NCHW')) \
            + b_res[None, :, None, None]

        h = jax.nn.relu(res + tco)
        hp = jnp.transpose(h, (0, 3, 2, 1))
        mu = jnp.mean(hp, axis=-1, keepdims=True)
        var = jnp.mean(jnp.square(hp - mu), axis=-1, keepdims=True)
        ln = gamma * (hp - mu) * jax.lax.rsqrt(var + EPS) + beta
        return jnp.transpose(ln, (0, 2, 3, 1))

    pspec_x = P("x")          # shard batch dim
    pspec_rep = P()           # replicated params
    in_specs = (pspec_x,) + (pspec_rep,) * 18
    fn = jax.jit(
        shard_map(block, mesh=mesh, in_specs=in_specs, out_specs=pspec_x)
    )
    _cache["fn"] = fn
    return fn


def kernel(x, cheb, U1, U2, U3, b_e, V_e, W1, W2, W3, b_s, V_s,
           Theta, W_time, b_time, W_res, b_res, gamma, beta):
    import jax.numpy as jnp

    fn = _get_compiled()
    args = [x, cheb, U1, U2, U3, b_e, V_e, W1, W2, W3, b_s, V_s,
            Theta, W_time, b_time, W_res, b_res, gamma, beta]
    args = [jnp.asarray(np.asarray(a), jnp.float32) for a in args]
    out = fn(*args)
    return np.asarray(out, dtype=np.float32)
